# revision 4
# baseline (speedup 1.0000x reference)
"""AnyPrecisionLinear (4-bit bitplane LUT quantized linear) on 8 TRN2 NeuronCores.

y = clip(x @ dequant(qweight, lut).T + bias)  with
  x: (8, 2048, 4096) fp16, qweight: (4, 4096, 128) int32 bitplanes,
  lut: (4096, 16) fp16 per-channel tables, bias: (4096,) fp16.

Sharding: column-parallel. Each core owns 512 output channels: it receives the
full x (replicated), its qweight/lut/bias shard, dequantizes its weight shard
on-device, computes y_shard = x @ W_shard.T + bias_shard, and writes the shard
feature-major (512, 16384). The host concatenates and transposes.

On-device dequant: per output channel the 16-entry LUT lookup is evaluated as
an exact bicubic polynomial in (h, l) = (code>>2, code&3) around symmetric
nodes {-1.5,-0.5,0.5,1.5}; coefficients (one 16-vector per channel) are
computed on-device from the lut via a constant 16x16 matrix. Bitplane words
are expanded with masked compares on the vector engine.
"""

import sys

import numpy as np

sys.path.insert(0, "/opt/trn_rl_repo")

import concourse.bass as bass
import concourse.tile as tile
from concourse import bacc, mybir
from concourse.bass_utils import run_bass_kernel_spmd

# problem shapes
T_FULL = 16384
I_F = 4096
O_F = 4096
N_CORES = 8
OS = O_F // N_CORES       # 512 output channels per core
W_BITS = 4

# tiling
KP = 128                  # contraction chunk (SBUF partitions)
NK = I_F // KP            # 32 k-chunks
OCN = OS // 128           # 4 o-chunks of 128 per core
TM = 1024                 # tokens per PSUM group (4 oc x 2 banks = 8 banks)
DQF = 512                 # dequant i-block (bits per block)
NIC = I_F // DQF          # 8 dequant blocks
HPW = DQF // 16           # uint16 halfwords per dequant block (32)

F16 = mybir.dt.float16
F32 = mybir.dt.float32
U16 = mybir.dt.uint16
ALU = mybir.AluOpType
ACTF = mybir.ActivationFunctionType


def _interp_matrix() -> np.ndarray:
    """16x16 fp32 M with coef[o, 4j+k] = sum_c lut[o, c] * M[c, 4j+k] giving
    P(h', l') = sum_jk coef_jk h'^j l'^k == lut[o, 4h+l], h' = h-1.5, l' = l-1.5."""
    nodes = np.array([-1.5, -0.5, 0.5, 1.5])
    V = np.stack([nodes**j for j in range(4)], axis=1)  # (4 nodes, 4 powers)
    Vinv = np.linalg.inv(V)
    M = np.zeros((16, 16))
    for c in range(16):
        h, l = c >> 2, c & 3
        for j in range(4):
            for k in range(4):
                M[c, 4 * j + k] = Vinv[j, h] * Vinv[k, l]
    return M.astype(np.float32)


def build_nc(t_tokens: int = T_FULL):
    assert t_tokens % TM == 0
    ng = t_tokens // TM
    nc = bacc.Bacc(None, target_bir_lowering=False, debug=False,
                   num_devices=N_CORES, enable_partition_id=False)

    x_ext = nc.declare_dram_parameter("x", [t_tokens, I_F], F16, isOutput=False)
    qw_ext = nc.declare_dram_parameter("qw16", [W_BITS, OS, I_F // 16], U16, isOutput=False)
    lut_ext = nc.declare_dram_parameter("lut", [OS, 16], F16, isOutput=False)
    b_ext = nc.declare_dram_parameter("bias", [OS, 1], F16, isOutput=False)
    mask_ext = nc.declare_dram_parameter("maskpat", [128, 16], U16, isOutput=False)
    minv_ext = nc.declare_dram_parameter("minv", [128, 256], F32, isOutput=False)
    out_ext = nc.declare_dram_parameter("out", [OS, t_tokens], F16, isOutput=True)

    with tile.TileContext(nc) as tc:
        with (
            tc.tile_pool(name="const", bufs=1) as constp,
            tc.tile_pool(name="qw", bufs=1) as qwp,
            tc.tile_pool(name="coef", bufs=1) as coefp,
            tc.tile_pool(name="dq", bufs=2) as dqp,
            tc.tile_pool(name="wt", bufs=1) as wtp,
            tc.tile_pool(name="xt", bufs=6) as xtp,
            tc.tile_pool(name="ot", bufs=8) as otp,
            tc.tile_pool(name="wdram", bufs=1, space="DRAM") as wdp,
            tc.tile_pool(name="psum", bufs=1, space="PSUM") as psp,
        ):
            # ---- small constant loads ----
            maskt = constp.tile([128, 16], U16)
            nc.scalar.dma_start(maskt[:], mask_ext[:])
            minvt = constp.tile([128, 256], F32)
            nc.scalar.dma_start(minvt[:], minv_ext[:])

            lut16 = constp.tile([128, 16 * OCN], F16, name="lut16")
            lut32 = constp.tile([128, 16 * OCN], F32, name="lut32")
            bias32 = constp.tile([128, OCN], F32, name="bias32")
            b16 = constp.tile([128, OCN], F16, name="b16")
            lut_r = lut_ext[:].rearrange("(a p) c -> p a c", p=128)
            nc.scalar.dma_start(lut16[:].rearrange("p (a c) -> p a c", c=16), lut_r)
            nc.vector.tensor_copy(lut32[:], lut16[:])
            bias_r = b_ext[:].rearrange("(a p) c -> p a c", p=128)
            nc.scalar.dma_start(b16[:].rearrange("p (a c) -> p a c", c=1), bias_r)
            nc.vector.tensor_copy(bias32[:], b16[:])

            # ---- per-o-chunk coefficient tables: coef = lut @ M ----
            coefs = []
            for oc in range(OCN):
                cf0 = coefp.tile([128, 16], F32, name=f"cf{oc}a")
                cf1 = coefp.tile([128, 16], F32, name=f"cf{oc}b")
                nc.vector.memset(cf0[:], 0.0)
                cur, nxt = cf0, cf1
                for c in range(16):
                    nc.vector.scalar_tensor_tensor(
                        out=nxt[:], in0=minvt[:, c * 16:(c + 1) * 16],
                        scalar=lut32[:, oc * 16 + c: oc * 16 + c + 1],
                        in1=cur[:], op0=ALU.mult, op1=ALU.add)
                    cur, nxt = nxt, cur
                coefs.append(cur)

            # ---- qweight plane tiles (o-chunk, plane) ----
            qts = {}
            for oc in range(OCN):
                for p in range(W_BITS):
                    qt = qwp.tile([128, I_F // 16], U16, name=f"q{oc}_{p}")
                    nc.scalar.dma_start(qt[:], qw_ext[p, oc * 128:(oc + 1) * 128, :])
                    qts[(oc, p)] = qt

            # ---- dequant: W (o, i) blocks -> DRAM wd blocks ----
            wds = []
            for ic in range(NIC):
                wd = wdp.tile([OS, DQF], F16, name=f"wd{ic}", tag=f"wd{ic}")
                wds.append(wd)

            for ic in range(NIC):
                for oc in range(OCN):
                    B = lambda j, k: coefs[oc][:, 4 * j + k: 4 * j + k + 1]
                    h0 = ic * HPW
                    mask3 = maskt[:].unsqueeze(1).broadcast_to((128, HPW, 16))
                    m = []
                    for p in range(W_BITS):
                        qexp = (qts[(oc, p)][:, h0:h0 + HPW]
                                .unsqueeze(2).broadcast_to((128, HPW, 16)))
                        mp = dqp.tile([128, DQF], U16, name=f"m{p}", tag=f"m{p}")
                        nc.vector.tensor_tensor(
                            out=mp[:].rearrange("q (a b) -> q a b", b=16),
                            in0=qexp, in1=mask3, op=ALU.bitwise_and)
                        m.append(mp)
                    # digits shifted to symmetric nodes: hp = 2*b0 + b1 - 1.5
                    th = dqp.tile([128, DQF], F16, name="th", tag="th")
                    nc.vector.tensor_scalar(out=th[:], in0=m[0][:], scalar1=0.0,
                                            scalar2=2.0, op0=ALU.not_equal, op1=ALU.mult)
                    hp = dqp.tile([128, DQF], F16, name="hp", tag="hp")
                    nc.vector.scalar_tensor_tensor(out=hp[:], in0=m[1][:], scalar=0.0,
                                                   in1=th[:], op0=ALU.not_equal, op1=ALU.add)
                    nc.vector.tensor_scalar(out=hp[:], in0=hp[:], scalar1=-1.5,
                                            scalar2=None, op0=ALU.add)
                    tl = dqp.tile([128, DQF], F16, name="tl", tag="tl")
                    nc.vector.tensor_scalar(out=tl[:], in0=m[2][:], scalar1=0.0,
                                            scalar2=2.0, op0=ALU.not_equal, op1=ALU.mult)
                    lp = dqp.tile([128, DQF], F16, name="lp", tag="lp")
                    nc.vector.scalar_tensor_tensor(out=lp[:], in0=m[3][:], scalar=0.0,
                                                   in1=tl[:], op0=ALU.not_equal, op1=ALU.add)
                    nc.vector.tensor_scalar(out=lp[:], in0=lp[:], scalar1=-1.5,
                                            scalar2=None, op0=ALU.add)
                    # cubics in l' (w_j = R_j - b_j0), fp32 intermediates
                    ws = []
                    for j in range(4):
                        u = dqp.tile([128, DQF], F16, name=f"u{j}", tag=f"u{j}")
                        nc.vector.tensor_scalar(out=u[:], in0=lp[:], scalar1=B(j, 3),
                                                scalar2=B(j, 2), op0=ALU.mult, op1=ALU.add)
                        v = dqp.tile([128, DQF], F16, name=f"v{j}", tag=f"v{j}")
                        nc.vector.scalar_tensor_tensor(out=v[:], in0=u[:], scalar=0.0,
                                                       in1=lp[:], op0=ALU.add, op1=ALU.mult)
                        w = dqp.tile([128, DQF], F16, name=f"w{j}", tag=f"w{j}")
                        nc.vector.scalar_tensor_tensor(out=w[:], in0=v[:], scalar=B(j, 1),
                                                       in1=lp[:], op0=ALU.add, op1=ALU.mult)
                        ws.append(w)
                    # Horner in h'
                    acc = dqp.tile([128, DQF], F16, name="acc", tag="acc")
                    acc2 = dqp.tile([128, DQF], F16, name="acc2", tag="acc2")
                    wblk = dqp.tile([128, DQF], F16, name="wblk", tag="wblk")
                    nc.vector.tensor_scalar(out=acc[:], in0=ws[3][:], scalar1=B(3, 0),
                                            scalar2=None, op0=ALU.add)
                    for j in (2, 1, 0):
                        nc.vector.scalar_tensor_tensor(out=acc2[:], in0=acc[:], scalar=0.0,
                                                       in1=hp[:], op0=ALU.add, op1=ALU.mult)
                        dst = wblk if j == 0 else acc
                        nc.vector.scalar_tensor_tensor(out=dst[:], in0=acc2[:], scalar=B(j, 0),
                                                       in1=ws[j][:], op0=ALU.add, op1=ALU.add)
                    nc.scalar.dma_start(wds[ic][oc * 128:(oc + 1) * 128, :], wblk[:])

            # ---- W^T tiles via DMA-transpose roundtrip ----
            wt = wtp.tile([128, NK * 512], F16, name="wt")
            for k in range(NK):
                ic = (k * KP) // DQF
                off = (k * KP) % DQF
                nc.sync.dma_start_transpose(
                    wt[:, k * 512:(k + 1) * 512],
                    wds[ic][0:OS, off:off + KP])

            # ---- main matmul loops ----
            for g in range(ng):
                pss = []
                for oc in range(OCN):
                    ps = psp.tile([128, TM], F32, name=f"ps{oc}", tag=f"ps{oc}")
                    pss.append(ps)
                for k in range(NK):
                    xt = xtp.tile([128, TM], F16, name="xt", tag="xt")
                    nc.sync.dma_start_transpose(
                        xt[:], x_ext[g * TM:(g + 1) * TM, k * KP:(k + 1) * KP])
                    for oc in range(OCN):
                        lhsT = wt[:, k * 512 + oc * 128: k * 512 + (oc + 1) * 128]
                        nc.tensor.matmul(pss[oc][:, 0:512], lhsT, xt[:, 0:512],
                                         start=(k == 0), stop=(k == NK - 1))
                        nc.tensor.matmul(pss[oc][:, 512:TM], lhsT, xt[:, 512:TM],
                                         start=(k == 0), stop=(k == NK - 1))
                for oc in range(OCN):
                    ot = otp.tile([128, TM], F16, name="ot", tag="ot")
                    nc.scalar.activation(ot[:], pss[oc][:], ACTF.Identity,
                                         bias=bias32[:, oc:oc + 1], scale=1.0)
                    nc.scalar.dma_start(
                        out_ext[oc * 128:(oc + 1) * 128, g * TM:(g + 1) * TM], ot[:])

    nc.compile()
    return nc


def make_in_maps(x2d: np.ndarray, qweight: np.ndarray, lut: np.ndarray,
                 bias: np.ndarray):
    """Shard inputs for the 8 cores. x2d: (t, 4096) fp16 row-major."""
    maskpat = np.ascontiguousarray(
        np.broadcast_to(
            (np.uint16(1) << np.arange(16, dtype=np.uint16))[None, :], (128, 16)))
    minv = np.ascontiguousarray(
        np.broadcast_to(_interp_matrix().reshape(1, 256), (128, 256)))
    in_maps = []
    for c in range(N_CORES):
        sl = slice(c * OS, (c + 1) * OS)
        qw16 = np.ascontiguousarray(qweight[:, sl, :]).view(np.uint16)
        in_maps.append({
            "x": x2d,
            "qw16": qw16,
            "lut": np.ascontiguousarray(lut[sl]),
            "bias": np.ascontiguousarray(bias[sl]).reshape(OS, 1),
            "maskpat": maskpat,
            "minv": minv,
        })
    return in_maps


_HOOK_DONE = [False]


def _install_ntff_hook():
    """The agent image's antenv lacks axon_hooks; shim it so trace=True
    can collect NTFF profiles via the boot's ctypes path."""
    if _HOOK_DONE[0]:
        return
    _HOOK_DONE[0] = True
    try:
        import types
        import antenv
        mod = types.ModuleType("antenv.axon_hooks")
        _hook = [None]
        mod.set_axon_ntff_profile_hook = lambda h: _hook.__setitem__(0, h)
        mod.get_axon_ntff_profile_hook = lambda: _hook[0]
        sys.modules["antenv.axon_hooks"] = mod
        antenv.axon_hooks = mod
        from trn_agent_boot.trn_boot import _ntff_profile_via_ctypes
        mod.set_axon_ntff_profile_hook(
            _ntff_profile_via_ctypes("/opt/axon/libaxon_pjrt.so"))
    except Exception as e:  # degrade to no tracing
        print(f"ntff hook install failed: {e}", file=sys.stderr)


_NC_CACHE: dict = {}


def _get_nc(t_tokens: int):
    if t_tokens not in _NC_CACHE:
        _NC_CACHE[t_tokens] = build_nc(t_tokens)
    return _NC_CACHE[t_tokens]


def run(x, qweight, lut, bias, trace: bool = False, **trace_kwargs):
    """Run on hardware; returns (y_full, BassKernelResults)."""
    xs = x.shape
    x2d = np.ascontiguousarray(np.asarray(x).reshape(-1, I_F))
    t_tokens = x2d.shape[0]
    if trace:
        _install_ntff_hook()
    nc = _get_nc(t_tokens)
    in_maps = make_in_maps(x2d, np.asarray(qweight), np.asarray(lut),
                           np.asarray(bias))
    res = run_bass_kernel_spmd(nc, in_maps, core_ids=list(range(N_CORES)),
                               trace=trace, **trace_kwargs)
    yT = np.empty((O_F, t_tokens), np.float16)
    for c in range(N_CORES):
        yT[c * OS:(c + 1) * OS] = res.results[c]["out"]
    y = np.ascontiguousarray(yT.T).reshape(xs[0], xs[1], O_F)
    return y, res


def kernel(x, qweight, lut, bias):
    y, _ = run(x, qweight, lut, bias)
    return y
